# revision 5
# baseline (speedup 1.0000x reference)
"""Trainium2 Bass kernel for nn_Cal_adj_matrix (pyramid-pool adjacency).

Computes, per sample b:
    feature = x[b].reshape(C, M)                  # M = H*W = 9216
    pool    = pyramid_pool(x[b])                  # (C, 50), pools of size 1,2,3,6
    sim     = relu(feature^T @ pool / (B*C*H*W))  # (M, 50)
    total   = sim.sum(-1)                         # (M,)
    adj     = sim / (total^2 + 1e-6)              # (M, 50)

Sharding: data-parallel over batch; 32 samples -> 4 per core x 8 cores.
"""

import numpy as np

import concourse.bass as bass
import concourse.bacc as bacc
import concourse.mybir as mybir
import concourse.tile as tile
from concourse.bass_utils import run_bass_kernel_spmd

# Problem shape (hardcoded; kernel.py must be self-contained).
B, C, H, W = 32, 256, 96, 96
M = H * W            # 9216
N = 50               # 1 + 4 + 9 + 36 pyramid tokens
NCORES = 8
BS = B // NCORES     # 4 samples per core
DIV = float(B * C * H * W)  # reference's global divisor

FP32 = mybir.dt.float32
BF16 = mybir.dt.bfloat16

# m-index mapping: m = p*72 + j  (p = PSUM partition, j = matmul index).
# This makes each sample's output one fully-contiguous (128 x 14400B) DMA.
JN = M // 128        # 72 matmul column-groups per sample
NQ = 4               # input DMA split: 4 quarters of 24 h-rows each
QH = H // NQ         # 24
QM = QH * W          # 2304 elements per quarter
R1Q = QH * 6         # 144 stage-1 outputs per quarter

BANK_J = 9           # matmul groups per PSUM bank (9*50=450 <= 512)
NBANK = JN // BANK_J  # 8 bank groups per sample


def build_nc():
    nc = bacc.Bacc(
        "TRN2",
        target_bir_lowering=False,
        debug=False,
        enable_asserts=True,
        num_devices=NCORES,
    )
    x = nc.dram_tensor("x", [BS, C, H, W], FP32, kind="ExternalInput").ap()
    out = nc.dram_tensor("out", [BS, M, N], FP32, kind="ExternalOutput").ap()

    # scale factors folded into the pool values: 1/(bin_elems * DIV)
    k1 = 1.0 / (9216.0 * DIV)
    k2 = 1.0 / (2304.0 * DIV)
    k3 = 1.0 / (1024.0 * DIV)
    k6 = 1.0 / (256.0 * DIV)

    with tile.TileContext(nc) as tc:
        with (
            tc.tile_pool(name="xq", bufs=4) as xq_pool,
            tc.tile_pool(name="featbf", bufs=4) as feat_pool,
            tc.tile_pool(name="r1", bufs=4) as r1_pool,
            tc.tile_pool(name="pools", bufs=8) as small_pool,
            tc.tile_pool(name="poolbf", bufs=4) as poolbf_pool,
            tc.tile_pool(name="outb", bufs=2) as outb_pool,
            tc.tile_pool(name="stats", bufs=2) as stats_pool,
            tc.tile_pool(name="psum", bufs=8, space="PSUM") as psum_pool,
        ):
            for s in range(BS):
                featbf = []
                poolbf = []
                for ch in range(2):
                    c0 = ch * 128
                    fb = feat_pool.tile([128, M], BF16, tag="featbf")
                    r1 = r1_pool.tile([128, 576], FP32, tag="r1")
                    for q in range(NQ):
                        t32 = xq_pool.tile([128, QM], FP32, tag="xq")
                        src = x[s, c0:c0 + 128, q * QH:(q + 1) * QH, :]
                        nc.sync.dma_start(out=t32[:], in_=src.rearrange("c h w -> c (h w)"))
                        # fp32 -> bf16 cast on ScalarE
                        nc.scalar.copy(fb[:, q * QM:(q + 1) * QM], t32[:])
                        # stage-1 pool: sum 16 contiguous w-elements
                        nc.vector.reduce_sum(
                            r1[:, q * R1Q:(q + 1) * R1Q],
                            fb[:, q * QM:(q + 1) * QM].rearrange("p (g k) -> p g k", k=16),
                            axis=mybir.AxisListType.X,
                        )
                    # stage-2: A[hb,wb] = 16x16 block sums.  r1 free idx = h*6+wb,
                    # h = hb*16+hh  ->  idx = hb*96 + hh*6 + wb
                    A = small_pool.tile([128, 36], FP32, tag="A")
                    nc.vector.reduce_sum(
                        A[:, :],
                        r1[:, :576].rearrange("p (hb hh wb) -> p hb wb hh", hb=6, hh=16, wb=6),
                        axis=mybir.AxisListType.X,
                    )
                    # s=3 pools: 2x2 groups of A blocks
                    Bt = small_pool.tile([128, 18], FP32, tag="B")  # [hb:6, wp:3]
                    # A idx = hb*6 + wb, wb = wp*2 + t -> idx = hb*6 + wp*2 + t
                    a2 = A[:, :36].rearrange("p (hb wp t) -> p t hb wp", hb=6, wp=3, t=2)
                    nc.vector.tensor_add(Bt[:, :], a2[:, 0, :], a2[:, 1, :])
                    s3raw = small_pool.tile([128, 9], FP32, tag="s3")
                    b2 = Bt[:, :18].rearrange("p (hp t wp) -> p t hp wp", hp=3, t=2, wp=3)
                    nc.vector.tensor_add(s3raw[:, :], b2[:, 0, :], b2[:, 1, :])
                    # s=2 pools: 3x3 groups of A blocks
                    Ct = small_pool.tile([128, 12], FP32, tag="C")  # [hb:6, wq:2]
                    nc.vector.reduce_sum(
                        Ct[:, :],
                        A[:, :36].rearrange("p (hb wq wt) -> p (hb wq) wt", hb=6, wq=2, wt=3),
                        axis=mybir.AxisListType.X,
                    )
                    s2raw = small_pool.tile([128, 4], FP32, tag="s2")
                    nc.vector.reduce_sum(
                        s2raw[:, :],
                        Ct[:, :12].rearrange("p (hq ht wq) -> p hq wq ht", hq=2, ht=3, wq=2),
                        axis=mybir.AxisListType.X,
                    )
                    # s=1 pool
                    s1raw = small_pool.tile([128, 1], FP32, tag="s1")
                    nc.vector.reduce_sum(s1raw[:, :], A[:, :36], axis=mybir.AxisListType.X)

                    pb = poolbf_pool.tile([128, N], BF16, tag="poolbf")
                    nc.vector.tensor_scalar_mul(pb[:, 0:1], s1raw[:, :], k1)
                    nc.vector.tensor_scalar_mul(pb[:, 1:5], s2raw[:, :], k2)
                    nc.vector.tensor_scalar_mul(pb[:, 5:14], s3raw[:, :], k3)
                    nc.vector.tensor_scalar_mul(pb[:, 14:50], A[:, :], k6)

                    featbf.append(fb)
                    poolbf.append(pb)

                # main matmuls: out[p, j*50+n] = sum_c feat[c, p*72+j] * pool[c, n]
                outb = outb_pool.tile([128, JN * N], FP32, tag="outb")
                for g in range(NBANK):
                    ps = psum_pool.tile([128, BANK_J * N], FP32, tag="ps")
                    for k in range(BANK_J):
                        j = g * BANK_J + k
                        for ch in range(2):
                            nc.tensor.matmul(
                                ps[:, k * N:(k + 1) * N],
                                featbf[ch][:, j:j + JN * 127 + 1:JN],
                                poolbf[ch][:, :],
                                start=(ch == 0),
                                stop=(ch == 1),
                            )
                    # relu PSUM -> SBUF
                    nc.scalar.activation(
                        outb[:, g * BANK_J * N:(g + 1) * BANK_J * N],
                        ps[:, :],
                        mybir.ActivationFunctionType.Relu,
                    )

                # row stats: total over each group of 50
                total = stats_pool.tile([128, JN], FP32, tag="total")
                nc.vector.reduce_sum(
                    total[:, :],
                    outb[:, :].rearrange("p (j n) -> p j n", n=N),
                    axis=mybir.AxisListType.X,
                )
                sq = stats_pool.tile([128, JN], FP32, tag="sq")
                nc.vector.tensor_mul(sq[:, :], total[:, :], total[:, :])
                nc.vector.tensor_scalar_add(sq[:, :], sq[:, :], 1e-6)
                scale = stats_pool.tile([128, JN], FP32, tag="scale")
                nc.vector.reciprocal(scale[:, :], sq[:, :])

                # adj = relu(sim) * scale, broadcast scale over the 50 tokens
                for j in range(JN):
                    nc.vector.tensor_scalar_mul(
                        outb[:, j * N:(j + 1) * N],
                        outb[:, j * N:(j + 1) * N],
                        scale[:, j:j + 1],
                    )

                # one contiguous output DMA per sample (m = p*72 + j)
                nc.sync.dma_start(
                    out=out[s].rearrange("(p j) n -> p (j n)", p=128),
                    in_=outb[:, :],
                )

    nc.compile()
    return nc


_NC_CACHE = None


def kernel(**inputs) -> np.ndarray:
    global _NC_CACHE
    x = np.ascontiguousarray(np.asarray(inputs["x"], dtype=np.float32))
    assert x.shape == (B, C, H, W)
    if _NC_CACHE is None:
        _NC_CACHE = build_nc()
    nc = _NC_CACHE
    in_maps = [{"x": x[i * BS:(i + 1) * BS]} for i in range(NCORES)]
    res = run_bass_kernel_spmd(nc, in_maps, list(range(NCORES)))
    outs = [res.results[i]["out"] for i in range(NCORES)]
    return np.concatenate(outs, axis=0).astype(np.float32)


if __name__ == "__main__":
    xt = np.random.randn(B, C, H, W).astype(np.float32)
    y = kernel(x=xt)
    print(y.shape, y.dtype)


# revision 7
# speedup vs baseline: 243.1490x; 243.1490x over previous
"""Trainium2 Bass kernel for nn_Cal_adj_matrix (pyramid-pool adjacency).

Computes, per sample b:
    feature = x[b].reshape(C, M)                  # M = H*W = 9216
    pool    = pyramid_pool(x[b])                  # (C, 50), pools of size 1,2,3,6
    sim     = relu(feature^T @ pool / (B*C*H*W))  # (M, 50)
    total   = sim.sum(-1)                         # (M,)
    adj     = sim / (total^2 + 1e-6)              # (M, 50)

Sharding: data-parallel over batch; 32 samples -> 4 per core x 8 cores.
"""

import numpy as np

import concourse.bass as bass
import concourse.bacc as bacc
import concourse.mybir as mybir
import concourse.tile as tile
from concourse.bass_utils import run_bass_kernel_spmd

# Problem shape (hardcoded; kernel.py must be self-contained).
B, C, H, W = 32, 256, 96, 96
M = H * W            # 9216
N = 50               # 1 + 4 + 9 + 36 pyramid tokens
NCORES = 8
BS = B // NCORES     # 4 samples per core
DIV = float(B * C * H * W)  # reference's global divisor

FP32 = mybir.dt.float32
BF16 = mybir.dt.bfloat16

# m-index mapping: m = p*72 + j  (p = PSUM partition, j = matmul index).
# This makes each sample's output one fully-contiguous (128 x 14400B) DMA.
JN = M // 128        # 72 matmul column-groups per sample
NQ = 4               # input DMA split: 4 quarters of 24 h-rows each
QH = H // NQ         # 24
QM = QH * W          # 2304 elements per quarter
R1Q = QH * 6         # 144 stage-1 outputs per quarter

BANK_J = 9           # matmul groups per PSUM bank (9*50=450 <= 512)
NBANK = JN // BANK_J  # 8 bank groups per sample


def build_nc(reps=1):
    nc = bacc.Bacc(
        "TRN2",
        target_bir_lowering=False,
        debug=False,
        enable_asserts=True,
        num_devices=NCORES,
    )
    x = nc.dram_tensor("x", [BS, C, H, W], FP32, kind="ExternalInput").ap()
    out = nc.dram_tensor("out", [BS, M, N], FP32, kind="ExternalOutput").ap()

    # scale factors folded into the pool values: 1/(bin_elems * DIV)
    k1 = 1.0 / (9216.0 * DIV)
    k2 = 1.0 / (2304.0 * DIV)
    k3 = 1.0 / (1024.0 * DIV)
    k6 = 1.0 / (256.0 * DIV)

    with tile.TileContext(nc) as tc:
        with (
            tc.tile_pool(name="xq", bufs=4) as xq_pool,
            tc.tile_pool(name="featbf", bufs=4) as feat_pool,
            tc.tile_pool(name="r1", bufs=4) as r1_pool,
            tc.tile_pool(name="pools", bufs=8) as small_pool,
            tc.tile_pool(name="poolbf", bufs=4) as poolbf_pool,
            tc.tile_pool(name="outb", bufs=2) as outb_pool,
            tc.tile_pool(name="stats", bufs=2) as stats_pool,
            tc.tile_pool(name="psum", bufs=8, space="PSUM") as psum_pool,
        ):
            for s in [s for _ in range(reps) for s in range(BS)]:
                featbf = []
                poolbf = []
                for ch in range(2):
                    c0 = ch * 128
                    fb = feat_pool.tile([128, M], BF16, tag="featbf")
                    r1 = r1_pool.tile([128, 576], FP32, tag="r1")
                    for q in range(NQ):
                        t32 = xq_pool.tile([128, QM], FP32, tag="xq")
                        src = x[s, c0:c0 + 128, q * QH:(q + 1) * QH, :]
                        nc.sync.dma_start(out=t32[:], in_=src.rearrange("c h w -> c (h w)"))
                        # fp32 -> bf16 cast on ScalarE
                        nc.scalar.copy(fb[:, q * QM:(q + 1) * QM], t32[:])
                        # stage-1 pool: sum 16 contiguous w-elements
                        nc.vector.reduce_sum(
                            r1[:, q * R1Q:(q + 1) * R1Q],
                            fb[:, q * QM:(q + 1) * QM].rearrange("p (g k) -> p g k", k=16),
                            axis=mybir.AxisListType.X,
                        )
                    # stage-2: A[hb,wb] = 16x16 block sums.  r1 free idx = h*6+wb,
                    # h = hb*16+hh  ->  idx = hb*96 + hh*6 + wb
                    A = small_pool.tile([128, 36], FP32, tag="A")
                    nc.vector.reduce_sum(
                        A[:, :],
                        r1[:, :576].rearrange("p (hb hh wb) -> p hb wb hh", hb=6, hh=16, wb=6),
                        axis=mybir.AxisListType.X,
                    )
                    # s=3 pools: 2x2 groups of A blocks
                    Bt = small_pool.tile([128, 18], FP32, tag="B")  # [hb:6, wp:3]
                    # A idx = hb*6 + wb, wb = wp*2 + t -> idx = hb*6 + wp*2 + t
                    a2 = A[:, :36].rearrange("p (hb wp t) -> p t hb wp", hb=6, wp=3, t=2)
                    nc.vector.tensor_add(Bt[:, :], a2[:, 0, :], a2[:, 1, :])
                    s3raw = small_pool.tile([128, 9], FP32, tag="s3")
                    b2 = Bt[:, :18].rearrange("p (hp t wp) -> p t hp wp", hp=3, t=2, wp=3)
                    nc.vector.tensor_add(s3raw[:, :], b2[:, 0, :], b2[:, 1, :])
                    # s=2 pools: 3x3 groups of A blocks
                    Ct = small_pool.tile([128, 12], FP32, tag="C")  # [hb:6, wq:2]
                    nc.vector.reduce_sum(
                        Ct[:, :],
                        A[:, :36].rearrange("p (hb wq wt) -> p (hb wq) wt", hb=6, wq=2, wt=3),
                        axis=mybir.AxisListType.X,
                    )
                    s2raw = small_pool.tile([128, 4], FP32, tag="s2")
                    nc.vector.reduce_sum(
                        s2raw[:, :],
                        Ct[:, :12].rearrange("p (hq ht wq) -> p hq wq ht", hq=2, ht=3, wq=2),
                        axis=mybir.AxisListType.X,
                    )
                    # s=1 pool
                    s1raw = small_pool.tile([128, 1], FP32, tag="s1")
                    nc.vector.reduce_sum(s1raw[:, :], A[:, :36], axis=mybir.AxisListType.X)

                    pb = poolbf_pool.tile([128, N], BF16, tag="poolbf")
                    nc.vector.tensor_scalar_mul(pb[:, 0:1], s1raw[:, :], k1)
                    nc.vector.tensor_scalar_mul(pb[:, 1:5], s2raw[:, :], k2)
                    nc.vector.tensor_scalar_mul(pb[:, 5:14], s3raw[:, :], k3)
                    nc.vector.tensor_scalar_mul(pb[:, 14:50], A[:, :], k6)

                    featbf.append(fb)
                    poolbf.append(pb)

                # main matmuls: out[p, j*50+n] = sum_c feat[c, p*72+j] * pool[c, n]
                outb = outb_pool.tile([128, JN * N], FP32, tag="outb")
                for g in range(NBANK):
                    ps = psum_pool.tile([128, BANK_J * N], FP32, tag="ps")
                    for k in range(BANK_J):
                        j = g * BANK_J + k
                        for ch in range(2):
                            nc.tensor.matmul(
                                ps[:, k * N:(k + 1) * N],
                                featbf[ch][:, j:j + JN * 127 + 1:JN],
                                poolbf[ch][:, :],
                                start=(ch == 0),
                                stop=(ch == 1),
                            )
                    # relu PSUM -> SBUF
                    nc.scalar.activation(
                        outb[:, g * BANK_J * N:(g + 1) * BANK_J * N],
                        ps[:, :],
                        mybir.ActivationFunctionType.Relu,
                    )

                # row stats: total over each group of 50
                total = stats_pool.tile([128, JN], FP32, tag="total")
                nc.vector.reduce_sum(
                    total[:, :],
                    outb[:, :].rearrange("p (j n) -> p j n", n=N),
                    axis=mybir.AxisListType.X,
                )
                sq = stats_pool.tile([128, JN], FP32, tag="sq")
                nc.vector.tensor_mul(sq[:, :], total[:, :], total[:, :])
                nc.vector.tensor_scalar_add(sq[:, :], sq[:, :], 1e-6)
                scale = stats_pool.tile([128, JN], FP32, tag="scale")
                nc.vector.reciprocal(scale[:, :], sq[:, :])

                # adj = relu(sim) * scale, broadcast scale over the 50 tokens
                for j in range(JN):
                    nc.vector.tensor_scalar_mul(
                        outb[:, j * N:(j + 1) * N],
                        outb[:, j * N:(j + 1) * N],
                        scale[:, j:j + 1],
                    )

                # one contiguous output DMA per sample (m = p*72 + j)
                nc.sync.dma_start(
                    out=out[s].rearrange("(p j) n -> p (j n)", p=128),
                    in_=outb[:, :],
                )

    nc.compile()
    return nc


_NC_CACHE = None


def kernel(**inputs) -> np.ndarray:
    global _NC_CACHE
    x = np.ascontiguousarray(np.asarray(inputs["x"], dtype=np.float32))
    assert x.shape == (B, C, H, W)
    if _NC_CACHE is None:
        _NC_CACHE = build_nc()
    nc = _NC_CACHE
    in_maps = [{"x": x[i * BS:(i + 1) * BS]} for i in range(NCORES)]
    res = run_bass_kernel_spmd(nc, in_maps, list(range(NCORES)))
    outs = [res.results[i]["out"] for i in range(NCORES)]
    return np.concatenate(outs, axis=0).astype(np.float32)


if __name__ == "__main__":
    xt = np.random.randn(B, C, H, W).astype(np.float32)
    y = kernel(x=xt)
    print(y.shape, y.dtype)


# revision 15
# speedup vs baseline: 251.5921x; 1.0347x over previous
"""Trainium2 Bass kernel for nn_Cal_adj_matrix (pyramid-pool adjacency).

Computes, per sample b:
    feature = x[b].reshape(C, M)                  # M = H*W = 9216
    pool    = pyramid_pool(x[b])                  # (C, 50), pools of size 1,2,3,6
    sim     = relu(feature^T @ pool / (B*C*H*W))  # (M, 50)
    total   = sim.sum(-1)                         # (M,)
    adj     = sim / (total^2 + 1e-6)              # (M, 50)

Sharding: data-parallel over batch; 32 samples -> 4 per core x 8 cores.
"""

import numpy as np

import concourse.bass as bass
import concourse.bacc as bacc
import concourse.mybir as mybir
import concourse.tile as tile
from concourse.bass_utils import run_bass_kernel_spmd

# Problem shape (hardcoded; kernel.py must be self-contained).
B, C, H, W = 32, 256, 96, 96
M = H * W            # 9216
N = 50               # 1 + 4 + 9 + 36 pyramid tokens
NCORES = 8
BS = B // NCORES     # 4 samples per core
DIV = float(B * C * H * W)  # reference's global divisor

FP32 = mybir.dt.float32
BF16 = mybir.dt.bfloat16

# m-index mapping: m = p*72 + j  (p = PSUM partition, j = matmul index).
# This makes each sample's output one fully-contiguous (128 x 14400B) DMA.
JN = M // 128        # 72 matmul column-groups per sample
NQ = 4               # input DMA split: 4 quarters of 24 h-rows each
QH = H // NQ         # 24
QM = QH * W          # 2304 elements per quarter
R1Q = QH * 6         # 144 stage-1 outputs per quarter

BANK_J = 9           # matmul groups per PSUM bank (9*50=450 <= 512)
NBANK = JN // BANK_J  # 8 bank groups per sample


def build_nc(reps=1):
    nc = bacc.Bacc(
        "TRN2",
        target_bir_lowering=False,
        debug=False,
        enable_asserts=True,
        num_devices=NCORES,
    )
    x = nc.dram_tensor("x", [BS, C, H, W], FP32, kind="ExternalInput").ap()
    out = nc.dram_tensor("out", [BS, M, N], FP32, kind="ExternalOutput").ap()

    # scale factors folded into the pool values: 1/(bin_elems * DIV)
    k1 = 1.0 / (9216.0 * DIV)
    k2 = 1.0 / (2304.0 * DIV)
    k3 = 1.0 / (1024.0 * DIV)
    k6 = 1.0 / (256.0 * DIV)

    with tile.TileContext(nc) as tc:
        with (
            tc.tile_pool(name="xq", bufs=4) as xq_pool,
            tc.tile_pool(name="featbf", bufs=4) as feat_pool,
            tc.tile_pool(name="r1", bufs=4) as r1_pool,
            tc.tile_pool(name="pools", bufs=8) as small_pool,
            tc.tile_pool(name="poolbf", bufs=4) as poolbf_pool,
            tc.tile_pool(name="outb", bufs=2) as outb_pool,
            tc.tile_pool(name="stats", bufs=2) as stats_pool,
            tc.tile_pool(name="psum", bufs=8, space="PSUM") as psum_pool,
        ):
            for s in [s for _ in range(reps) for s in range(BS)]:
                featbf = []
                poolbf = []
                for ch in range(2):
                    c0 = ch * 128
                    fb = feat_pool.tile([128, M], BF16, tag="featbf")
                    r1 = r1_pool.tile([128, 576], FP32, tag="r1")
                    for q in range(NQ):
                        t32 = xq_pool.tile([128, QM], FP32, tag="xq")
                        src = x[s, c0:c0 + 128, q * QH:(q + 1) * QH, :]
                        nc.sync.dma_start(out=t32[:], in_=src.rearrange("c h w -> c (h w)"))
                        # fp32 -> bf16 cast on ScalarE
                        nc.scalar.copy(fb[:, q * QM:(q + 1) * QM], t32[:])
                        # stage-1 pool: sum 16 contiguous w-elements (from
                        # the fp32 tile so it runs in parallel with the cast)
                        nc.vector.reduce_sum(
                            r1[:, q * R1Q:(q + 1) * R1Q],
                            t32[:, :].rearrange("p (g k) -> p g k", k=16),
                            axis=mybir.AxisListType.X,
                        )
                    # stage-2: A[hb,wb] = 16x16 block sums.  r1 free idx = h*6+wb,
                    # h = hb*16+hh  ->  idx = hb*96 + hh*6 + wb
                    A = small_pool.tile([128, 36], FP32, tag="A")
                    nc.vector.reduce_sum(
                        A[:, :],
                        r1[:, :576].rearrange("p (hb hh wb) -> p hb wb hh", hb=6, hh=16, wb=6),
                        axis=mybir.AxisListType.X,
                    )
                    # s=3 pools: 2x2 groups of A blocks
                    Bt = small_pool.tile([128, 18], FP32, tag="B")  # [hb:6, wp:3]
                    # A idx = hb*6 + wb, wb = wp*2 + t -> idx = hb*6 + wp*2 + t
                    a2 = A[:, :36].rearrange("p (hb wp t) -> p t hb wp", hb=6, wp=3, t=2)
                    nc.vector.tensor_add(Bt[:, :], a2[:, 0, :], a2[:, 1, :])
                    s3raw = small_pool.tile([128, 9], FP32, tag="s3")
                    b2 = Bt[:, :18].rearrange("p (hp t wp) -> p t hp wp", hp=3, t=2, wp=3)
                    nc.vector.tensor_add(s3raw[:, :], b2[:, 0, :], b2[:, 1, :])
                    # s=2 pools: 3x3 groups of A blocks
                    Ct = small_pool.tile([128, 12], FP32, tag="C")  # [hb:6, wq:2]
                    nc.vector.reduce_sum(
                        Ct[:, :],
                        A[:, :36].rearrange("p (hb wq wt) -> p (hb wq) wt", hb=6, wq=2, wt=3),
                        axis=mybir.AxisListType.X,
                    )
                    s2raw = small_pool.tile([128, 4], FP32, tag="s2")
                    nc.vector.reduce_sum(
                        s2raw[:, :],
                        Ct[:, :12].rearrange("p (hq ht wq) -> p hq wq ht", hq=2, ht=3, wq=2),
                        axis=mybir.AxisListType.X,
                    )
                    # s=1 pool
                    s1raw = small_pool.tile([128, 1], FP32, tag="s1")
                    nc.vector.reduce_sum(s1raw[:, :], A[:, :36], axis=mybir.AxisListType.X)

                    pb = poolbf_pool.tile([128, N], BF16, tag="poolbf")
                    nc.vector.tensor_scalar_mul(pb[:, 0:1], s1raw[:, :], k1)
                    nc.vector.tensor_scalar_mul(pb[:, 1:5], s2raw[:, :], k2)
                    nc.vector.tensor_scalar_mul(pb[:, 5:14], s3raw[:, :], k3)
                    nc.vector.tensor_scalar_mul(pb[:, 14:50], A[:, :], k6)

                    featbf.append(fb)
                    poolbf.append(pb)

                # main matmuls: out[p, j*50+n] = sum_c feat[c, p*72+j] * pool[c, n]
                # lo-half contributions emitted first so they can run while
                # the hi c-half is still streaming in.
                # NOTE: matmul start=True marks the whole 2KB PSUM bank
                # pending-zero, so accumulation groups sharing a bank must be
                # strictly sequential (start,stop adjacent per j).
                outb = outb_pool.tile([128, JN * N], FP32, tag="outb")
                for g in range(NBANK):
                    ps = psum_pool.tile([128, BANK_J * N], FP32, tag="ps")
                    for k in range(BANK_J):
                        j = g * BANK_J + k
                        for ch in range(2):
                            nc.tensor.matmul(
                                ps[:, k * N:(k + 1) * N],
                                featbf[ch][:, j:j + JN * 127 + 1:JN],
                                poolbf[ch][:, :],
                                start=(ch == 0),
                                stop=(ch == 1),
                            )
                    # relu PSUM -> SBUF
                    nc.scalar.activation(
                        outb[:, g * BANK_J * N:(g + 1) * BANK_J * N],
                        ps[:, :],
                        mybir.ActivationFunctionType.Relu,
                    )

                # per half: rowsum, scale = 1/(total^2+1e-6), multiply, DMA out
                # (halves drain earlier; out-DMAs ride the SWDGE ring so they
                # never head-of-line-block the input HWDGE ring)
                out_dram = out[s].rearrange("(p j) n -> p (j n)", p=128)
                NCHUNK = 4
                JH = JN // NCHUNK
                half = JH * N
                for hf in range(NCHUNK):
                    sl = slice(hf * half, (hf + 1) * half)
                    total = stats_pool.tile([128, JH], FP32, tag="total")
                    nc.vector.reduce_sum(
                        total[:, :],
                        outb[:, sl].rearrange("p (j n) -> p j n", n=N),
                        axis=mybir.AxisListType.X,
                    )
                    sq = stats_pool.tile([128, JH], FP32, tag="sq")
                    nc.vector.tensor_mul(sq[:, :], total[:, :], total[:, :])
                    nc.vector.tensor_scalar_add(sq[:, :], sq[:, :], 1e-6)
                    scale = stats_pool.tile([128, JH], FP32, tag="scale")
                    nc.vector.reciprocal(scale[:, :], sq[:, :])
                    nc.vector.tensor_mul(
                        outb[:, sl].rearrange("p (j n) -> p j n", n=N),
                        outb[:, sl].rearrange("p (j n) -> p j n", n=N),
                        scale[:, :].unsqueeze(2).broadcast_to((128, JH, N)),
                    )
                    # contiguous output DMA (m = p*72 + j)
                    nc.gpsimd.dma_start(out=out_dram[:, sl], in_=outb[:, sl])

    nc.compile()
    return nc


_NC_CACHE = None


def kernel(**inputs) -> np.ndarray:
    global _NC_CACHE
    x = np.ascontiguousarray(np.asarray(inputs["x"], dtype=np.float32))
    assert x.shape == (B, C, H, W)
    if _NC_CACHE is None:
        _NC_CACHE = build_nc()
    nc = _NC_CACHE
    in_maps = [{"x": x[i * BS:(i + 1) * BS]} for i in range(NCORES)]
    res = run_bass_kernel_spmd(nc, in_maps, list(range(NCORES)))
    outs = [res.results[i]["out"] for i in range(NCORES)]
    return np.concatenate(outs, axis=0).astype(np.float32)


if __name__ == "__main__":
    xt = np.random.randn(B, C, H, W).astype(np.float32)
    y = kernel(x=xt)
    print(y.shape, y.dtype)


# revision 19
# speedup vs baseline: 289.4214x; 1.1504x over previous
"""Trainium2 Bass kernel for nn_Cal_adj_matrix (pyramid-pool adjacency).

Computes, per sample b:
    feature = x[b].reshape(C, M)                  # M = H*W = 9216
    pool    = pyramid_pool(x[b])                  # (C, 50), pools of size 1,2,3,6
    sim     = relu(feature^T @ pool / (B*C*H*W))  # (M, 50)
    total   = sim.sum(-1)                         # (M,)
    adj     = sim / (total^2 + 1e-6)              # (M, 50)

Sharding: data-parallel over batch; 32 samples -> 4 per core x 8 cores.
"""

import numpy as np

import concourse.bass as bass
import concourse.bacc as bacc
import concourse.mybir as mybir
import concourse.tile as tile
from concourse.bass_utils import run_bass_kernel_spmd

# Problem shape (hardcoded; kernel.py must be self-contained).
B, C, H, W = 32, 256, 96, 96
M = H * W            # 9216
N = 50               # 1 + 4 + 9 + 36 pyramid tokens
NCORES = 8
BS = B // NCORES     # 4 samples per core
DIV = float(B * C * H * W)  # reference's global divisor

FP32 = mybir.dt.float32
BF16 = mybir.dt.bfloat16

# m-index mapping: m = p*72 + j  (p = PSUM partition, j = matmul index).
# This makes each sample's output one fully-contiguous (128 x 14400B) DMA.
JN = M // 128        # 72 matmul column-groups per sample
NQ = 4               # input DMA split: 4 quarters of 24 h-rows each
QH = H // NQ         # 24
QM = QH * W          # 2304 elements per quarter
R1Q = QH * 6         # 144 stage-1 outputs per quarter

BANK_J = 9           # matmul groups per PSUM bank (9*50=450 <= 512)
NBANK = JN // BANK_J  # 8 bank groups per sample


def build_nc(reps=1, xq_bufs=4, feat_bufs=4, outb_bufs=2):
    nc = bacc.Bacc(
        "TRN2",
        target_bir_lowering=False,
        debug=False,
        enable_asserts=True,
        num_devices=NCORES,
    )
    x = nc.dram_tensor("x", [BS, C, H, W], FP32, kind="ExternalInput").ap()
    out = nc.dram_tensor("out", [BS, M, N], FP32, kind="ExternalOutput").ap()

    # scale factors folded into the pool values: 1/(bin_elems * DIV)
    k1 = 1.0 / (9216.0 * DIV)
    k2 = 1.0 / (2304.0 * DIV)
    k3 = 1.0 / (1024.0 * DIV)
    k6 = 1.0 / (256.0 * DIV)

    with tile.TileContext(nc) as tc:
        with (
            tc.tile_pool(name="xq", bufs=xq_bufs) as xq_pool,
            tc.tile_pool(name="featbf", bufs=feat_bufs) as feat_pool,
            tc.tile_pool(name="r1", bufs=4) as r1_pool,
            tc.tile_pool(name="pools", bufs=8) as small_pool,
            tc.tile_pool(name="poolbf", bufs=4) as poolbf_pool,
            tc.tile_pool(name="outb", bufs=outb_bufs) as outb_pool,
            tc.tile_pool(name="stats", bufs=2) as stats_pool,
            tc.tile_pool(name="psum", bufs=8, space="PSUM") as psum_pool,
        ):
            for s in [s for _ in range(reps) for s in range(BS)]:
                featbf = []
                poolbf = []
                for ch in range(2):
                    c0 = ch * 128
                    fb = feat_pool.tile([128, M], BF16, tag="featbf")
                    r1 = r1_pool.tile([128, 576], FP32, tag="r1")
                    for q in range(NQ):
                        t32 = xq_pool.tile([128, QM], FP32, tag="xq")
                        src = x[s, c0:c0 + 128, q * QH:(q + 1) * QH, :]
                        nc.sync.dma_start(out=t32[:], in_=src.rearrange("c h w -> c (h w)"))
                        # fp32 -> bf16 cast on ScalarE
                        nc.scalar.copy(fb[:, q * QM:(q + 1) * QM], t32[:])
                        # stage-1 pool: sum 16 contiguous w-elements (from
                        # the fp32 tile so it runs in parallel with the cast)
                        nc.vector.reduce_sum(
                            r1[:, q * R1Q:(q + 1) * R1Q],
                            t32[:, :].rearrange("p (g k) -> p g k", k=16),
                            axis=mybir.AxisListType.X,
                        )
                    # stage-2: A[hb,wb] = 16x16 block sums.  r1 free idx = h*6+wb,
                    # h = hb*16+hh  ->  idx = hb*96 + hh*6 + wb
                    A = small_pool.tile([128, 36], FP32, tag="A")
                    nc.vector.reduce_sum(
                        A[:, :],
                        r1[:, :576].rearrange("p (hb hh wb) -> p hb wb hh", hb=6, hh=16, wb=6),
                        axis=mybir.AxisListType.X,
                    )
                    # s=3 pools: 2x2 groups of A blocks
                    Bt = small_pool.tile([128, 18], FP32, tag="B")  # [hb:6, wp:3]
                    # A idx = hb*6 + wb, wb = wp*2 + t -> idx = hb*6 + wp*2 + t
                    a2 = A[:, :36].rearrange("p (hb wp t) -> p t hb wp", hb=6, wp=3, t=2)
                    nc.vector.tensor_add(Bt[:, :], a2[:, 0, :], a2[:, 1, :])
                    s3raw = small_pool.tile([128, 9], FP32, tag="s3")
                    b2 = Bt[:, :18].rearrange("p (hp t wp) -> p t hp wp", hp=3, t=2, wp=3)
                    nc.vector.tensor_add(s3raw[:, :], b2[:, 0, :], b2[:, 1, :])
                    # s=2 pools: 3x3 groups of A blocks
                    Ct = small_pool.tile([128, 12], FP32, tag="C")  # [hb:6, wq:2]
                    nc.vector.reduce_sum(
                        Ct[:, :],
                        A[:, :36].rearrange("p (hb wq wt) -> p (hb wq) wt", hb=6, wq=2, wt=3),
                        axis=mybir.AxisListType.X,
                    )
                    s2raw = small_pool.tile([128, 4], FP32, tag="s2")
                    nc.vector.reduce_sum(
                        s2raw[:, :],
                        Ct[:, :12].rearrange("p (hq ht wq) -> p hq wq ht", hq=2, ht=3, wq=2),
                        axis=mybir.AxisListType.X,
                    )
                    # s=1 pool
                    s1raw = small_pool.tile([128, 1], FP32, tag="s1")
                    nc.vector.reduce_sum(s1raw[:, :], A[:, :36], axis=mybir.AxisListType.X)

                    pb = poolbf_pool.tile([128, N], BF16, tag="poolbf")
                    nc.vector.tensor_scalar_mul(pb[:, 0:1], s1raw[:, :], k1)
                    nc.vector.tensor_scalar_mul(pb[:, 1:5], s2raw[:, :], k2)
                    nc.vector.tensor_scalar_mul(pb[:, 5:14], s3raw[:, :], k3)
                    nc.vector.tensor_scalar_mul(pb[:, 14:50], A[:, :], k6)

                    featbf.append(fb)
                    poolbf.append(pb)

                # main matmuls: out[p, j*50+n] = sum_c feat[c, p*72+j] * pool[c, n]
                # lo-half contributions emitted first so they can run while
                # the hi c-half is still streaming in.
                # NOTE: matmul start=True marks the whole 2KB PSUM bank
                # pending-zero, so accumulation groups sharing a bank must be
                # strictly sequential (start,stop adjacent per j).
                outb = outb_pool.tile([128, JN * N], FP32, tag="outb")
                for g in range(NBANK):
                    ps = psum_pool.tile([128, BANK_J * N], FP32, tag="ps")
                    for k in range(BANK_J):
                        j = g * BANK_J + k
                        for ch in range(2):
                            nc.tensor.matmul(
                                ps[:, k * N:(k + 1) * N],
                                featbf[ch][:, j:j + JN * 127 + 1:JN],
                                poolbf[ch][:, :],
                                start=(ch == 0),
                                stop=(ch == 1),
                            )
                    # relu PSUM -> SBUF
                    nc.scalar.activation(
                        outb[:, g * BANK_J * N:(g + 1) * BANK_J * N],
                        ps[:, :],
                        mybir.ActivationFunctionType.Relu,
                    )

                # per half: rowsum, scale = 1/(total^2+1e-6), multiply, DMA out
                # (halves drain earlier; out-DMAs ride the SWDGE ring so they
                # never head-of-line-block the input HWDGE ring)
                out_dram = out[s].rearrange("(p j) n -> p (j n)", p=128)
                NCHUNK = 4
                JH = JN // NCHUNK
                half = JH * N
                for hf in range(NCHUNK):
                    sl = slice(hf * half, (hf + 1) * half)
                    total = stats_pool.tile([128, JH], FP32, tag="total")
                    nc.vector.reduce_sum(
                        total[:, :],
                        outb[:, sl].rearrange("p (j n) -> p j n", n=N),
                        axis=mybir.AxisListType.X,
                    )
                    sq = stats_pool.tile([128, JH], FP32, tag="sq")
                    nc.vector.tensor_mul(sq[:, :], total[:, :], total[:, :])
                    nc.vector.tensor_scalar_add(sq[:, :], sq[:, :], 1e-6)
                    scale = stats_pool.tile([128, JH], FP32, tag="scale")
                    nc.vector.reciprocal(scale[:, :], sq[:, :])
                    # on GpSimd: frees the loaded vector engine
                    nc.gpsimd.tensor_mul(
                        outb[:, sl].rearrange("p (j n) -> p j n", n=N),
                        outb[:, sl].rearrange("p (j n) -> p j n", n=N),
                        scale[:, :].unsqueeze(2).broadcast_to((128, JH, N)),
                    )
                    # contiguous output DMA (m = p*72 + j)
                    nc.gpsimd.dma_start(out=out_dram[:, sl], in_=outb[:, sl])

    nc.compile()
    return nc


_NC_CACHE = None


def kernel(**inputs) -> np.ndarray:
    global _NC_CACHE
    x = np.ascontiguousarray(np.asarray(inputs["x"], dtype=np.float32))
    assert x.shape == (B, C, H, W)
    if _NC_CACHE is None:
        _NC_CACHE = build_nc()
    nc = _NC_CACHE
    in_maps = [{"x": x[i * BS:(i + 1) * BS]} for i in range(NCORES)]
    res = run_bass_kernel_spmd(nc, in_maps, list(range(NCORES)))
    outs = [res.results[i]["out"] for i in range(NCORES)]
    return np.concatenate(outs, axis=0).astype(np.float32)


if __name__ == "__main__":
    xt = np.random.randn(B, C, H, W).astype(np.float32)
    y = kernel(x=xt)
    print(y.shape, y.dtype)
